# revision 1
# baseline (speedup 1.0000x reference)
"""Trainium2 Bass kernel for FullTensorProduct (64x0e+64x1o) x (1x0e+1x1o), uvuv.

Self-contained: accepts FULL inputs x1 (100000, 256) f32 and x2 (100000, 4)
f32, shards the edge dimension across 8 NeuronCores (pure data parallel),
runs one SPMD Bass kernel, and returns the FULL (100000, 1024) f32 output.

Per-core layout: 12500 edges mapped group-major as edge = t*125 + p over
125 SBUF partitions x 100 groups, so each supertile's x1/out DRAM region is
contiguous (strided per-partition HBM writes measured ~1.15x slower); x2 is
host-permuted to match. Supertiles of G=10 groups: one large DMA in and one
large DMA out per supertile, both via gpsimd (SWDGE) — measured to overlap
loads with stores, unlike the single sync HWDGE ring which serializes them.
v1 x v2 outer products, cross/quad combines, and o_sv run as broadcast
tensor_tensor ops on VectorE; o_ss and o_vs as per-group activation-scale
ops on ScalarE (kept to 2 ops/group — ScalarE costs ~1us per small
instruction on real HW). Measured 271.8us/core per pass (For_i slope).

Output columns (1024): [o_ss 0:64 | o_vv0 64:128 | o_sv 128:320 |
                        o_vs 320:512 | o_cross 512:704 | o_quad 704:1024]
"""

import numpy as np

import concourse.bass as bass
import concourse.bacc as bacc
import concourse.mybir as mybir
import concourse.tile as tile
from concourse.bass_utils import run_bass_kernel_spmd

F32 = mybir.dt.float32
INV_SQRT2 = float(1.0 / np.sqrt(2.0))
C_VV0 = float(np.sqrt(2.0) / np.sqrt(3.0))
C_Q2 = float(1.0 / np.sqrt(3.0))

N_CORES = 8
N_EDGES = 100000
ROWS_PER_CORE = N_EDGES // N_CORES  # 12500
P_PART = 125                        # 12500 = 125 * 100
G_GROUPS = 10


def _emit(nc: bass.Bass, rows: int, P: int, G: int):
    T = rows // P
    assert P * T == rows and T % G == 0
    n_super = T // G
    mult = mybir.AluOpType.mult
    subtract = mybir.AluOpType.subtract

    x1 = nc.dram_tensor("x1", (rows, 256), F32, kind="ExternalInput")
    x2 = nc.dram_tensor("x2", (rows, 4), F32, kind="ExternalInput")
    out = nc.dram_tensor("out", (rows, 1024), F32, kind="ExternalOutput")

    # group-major: edge = t*P + p, so every supertile's x1/out DRAM region
    # is one contiguous block (strided per-partition writes measurably halve
    # HBM write bandwidth). x2 arrives host-permuted to [p*T+t] order.
    X1 = x1[:].rearrange("(t p) c -> p t c", p=P)
    X2 = x2[:].rearrange("(p t) c -> p (t c)", p=P)
    OUT = out[:].rearrange("(t p) c -> p t c", p=P)

    with tile.TileContext(nc) as tc:
        with (
            tc.tile_pool(name="xin", bufs=3) as xin_pool,
            tc.tile_pool(name="outp", bufs=2) as out_pool,
            tc.tile_pool(name="prod", bufs=2) as prod_pool,
            tc.tile_pool(name="singles", bufs=1) as singles,
        ):
            x2t = singles.tile([P, T * 4], F32)
            x2s = singles.tile([P, T * 4], F32)
            nc.sync.dma_start(out=x2t[:], in_=X2)
            nc.vector.tensor_scalar_mul(out=x2s[:], in0=x2t[:], scalar1=INV_SQRT2)
            x2s3 = x2s[:].rearrange("p (t c) -> p t c", c=4)

            for s in range(n_super):
                t0 = s * G
                xt = xin_pool.tile([P, G, 256], F32, tag="x1t")
                nc.gpsimd.dma_start(out=xt[:], in_=X1[:, t0 : t0 + G, :])
                ot = out_pool.tile([P, G, 1024], F32, tag="outt")

                # products P_i[p,g,u,j] = v1[u,i] * v2[j] / sqrt2  (DVE)
                v1 = xt[:, :, 64:256].rearrange("p g (u i) -> p g u i", i=3)
                v2b = (
                    x2s3[:, t0 : t0 + G, 1:4]
                    .unsqueeze(2)
                    .broadcast_to((P, G, 64, 3))
                )
                Pt = []
                for i in range(3):
                    pt = prod_pool.tile([P, G, 64, 3], F32, tag=f"P{i}")
                    in0 = v1[:, :, :, i].unsqueeze(3).broadcast_to((P, G, 64, 3))
                    nc.vector.tensor_mul(out=pt[:], in0=in0, in1=v2b)
                    Pt.append(pt)

                def pij(i, j):
                    return Pt[i][:, :, :, j]

                # per-group scale paths: ss + vs on ScalarE (2 ops/group)
                for g in range(G):
                    tg = t0 + g
                    s2 = x2t[:, tg * 4 : tg * 4 + 1]
                    nc.scalar.mul(out=ot[:, g, 0:64], in_=xt[:, g, 0:64], mul=s2)
                    nc.scalar.mul(out=ot[:, g, 320:512], in_=xt[:, g, 64:256], mul=s2)

                # o_sv = s1 x v2 as one broadcast TT op on DVE
                x2r3 = x2t[:].rearrange("p (t c) -> p t c", c=4)
                svd = ot[:, :, 128:320].rearrange("p g (u j) -> p g u j", j=3)
                s1b = xt[:, :, 0:64].unsqueeze(3).broadcast_to((P, G, 64, 3))
                v2raw = (
                    x2r3[:, t0 : t0 + G, 1:4]
                    .unsqueeze(2)
                    .broadcast_to((P, G, 64, 3))
                )
                nc.vector.tensor_mul(out=svd, in0=s1b, in1=v2raw)

                # cross / quad / vv0 combines (DVE)
                crs = ot[:, :, 512:704].rearrange("p g (u k) -> p g u k", k=3)
                qd = ot[:, :, 704:1024].rearrange("p g (u m) -> p g u m", m=5)
                nc.vector.tensor_sub(out=crs[:, :, :, 0], in0=pij(1, 2), in1=pij(2, 1))
                nc.vector.tensor_sub(out=crs[:, :, :, 1], in0=pij(2, 0), in1=pij(0, 2))
                nc.vector.tensor_sub(out=crs[:, :, :, 2], in0=pij(0, 1), in1=pij(1, 0))
                nc.vector.tensor_add(out=qd[:, :, :, 0], in0=pij(0, 1), in1=pij(1, 0))
                nc.vector.tensor_add(out=qd[:, :, :, 1], in0=pij(1, 2), in1=pij(2, 1))
                nc.vector.tensor_add(out=qd[:, :, :, 3], in0=pij(0, 2), in1=pij(2, 0))
                nc.vector.tensor_sub(out=qd[:, :, :, 4], in0=pij(0, 0), in1=pij(1, 1))

                A = prod_pool.tile([P, G, 64], F32, tag="A")
                B = prod_pool.tile([P, G, 64], F32, tag="B")
                C = prod_pool.tile([P, G, 64], F32, tag="C")
                nc.vector.tensor_add(out=A[:], in0=pij(0, 0), in1=pij(1, 1))
                nc.vector.tensor_add(out=B[:], in0=A[:], in1=pij(2, 2))
                nc.vector.tensor_scalar_mul(out=ot[:, :, 64:128], in0=B[:], scalar1=C_VV0)
                nc.vector.scalar_tensor_tensor(
                    out=C[:], in0=pij(2, 2), scalar=3.0, in1=B[:],
                    op0=mult, op1=subtract,
                )
                nc.vector.tensor_scalar_mul(out=qd[:, :, :, 2], in0=C[:], scalar1=C_Q2)

                # both DMAs on gpsimd (SWDGE): measured to overlap loads and
                # stores fully, unlike the single sync HWDGE ring.
                nc.gpsimd.dma_start(out=OUT[:, t0 : t0 + G, :], in_=ot[:])
    return nc


_NC_CACHE = None


def _get_nc() -> bass.Bass:
    global _NC_CACHE
    if _NC_CACHE is None:
        nc = bacc.Bacc()
        _emit(nc, ROWS_PER_CORE, P_PART, G_GROUPS)
        nc.compile()   # bacc: reg alloc + split multi-sync-waits for TRN2
        nc.finalize()
        _NC_CACHE = nc
    return _NC_CACHE


def run(x1: np.ndarray, x2: np.ndarray, trace: bool = False):
    """Shard, run SPMD on 8 cores, gather. Returns (out, BassKernelResults)."""
    assert x1.shape == (N_EDGES, 256) and x2.shape == (N_EDGES, 4)
    x1 = np.ascontiguousarray(x1, dtype=np.float32)
    x2 = np.ascontiguousarray(x2, dtype=np.float32)
    nc = _get_nc()
    R = ROWS_PER_CORE
    T = R // P_PART
    perm = (np.arange(T)[None, :] * P_PART + np.arange(P_PART)[:, None]).reshape(-1)
    in_maps = [
        {"x1": x1[i * R : (i + 1) * R], "x2": x2[i * R : (i + 1) * R][perm]}
        for i in range(N_CORES)
    ]
    br = run_bass_kernel_spmd(nc, in_maps, list(range(N_CORES)), trace=trace)
    out = np.concatenate([br.results[i]["out"] for i in range(N_CORES)], axis=0)
    return out, br


def kernel(x1: np.ndarray, x2: np.ndarray) -> np.ndarray:
    out, _ = run(x1, x2, trace=False)
    return out


def make_timed_runner(nc=None, n_cores=N_CORES):
    """Build a shard_map-jitted callable over the 8 cores with device-resident
    inputs, mirroring bass2jax.run_bass_via_pjrt but without output donation,
    so repeated invocations measure device execution time.

    Returns (fn, dev_args): call fn(*dev_args) -> tuple of jax output arrays.
    """
    import jax
    import jax.numpy as jnp
    from jax.experimental.shard_map import shard_map
    from jax.sharding import Mesh, NamedSharding, PartitionSpec

    from concourse import bass2jax, mybir as mb

    bass2jax.install_neuronx_cc_hook()
    if nc is None:
        nc = _get_nc()
    assert nc.dbg_addr is None
    partition_name = nc.partition_id_tensor.name if nc.partition_id_tensor else None

    in_names, out_names, out_avals = [], [], []
    for alloc in nc.m.functions[0].allocations:
        if not isinstance(alloc, mb.MemoryLocationSet):
            continue
        name = alloc.memorylocations[0].name
        if alloc.kind == "ExternalInput":
            if name != partition_name:
                in_names.append(name)
        elif alloc.kind == "ExternalOutput":
            out_names.append(name)
            out_avals.append(
                jax.core.ShapedArray(tuple(alloc.tensor_shape), mb.dt.np(alloc.dtype))
            )
    n_params = len(in_names)
    all_names = in_names + out_names
    if partition_name is not None:
        all_names = all_names + [partition_name]

    def _body(*args):
        operands = list(args)
        if partition_name is not None:
            operands.append(bass2jax.partition_id_tensor())
        outs = bass2jax._bass_exec_p.bind(
            *operands,
            out_avals=tuple(out_avals),
            in_names=tuple(all_names),
            out_names=tuple(out_names),
            lowering_input_output_aliases=(),
            sim_require_finite=True,
            sim_require_nnan=True,
            nc=nc,
        )
        return tuple(outs)

    devices = jax.devices()[:n_cores]
    mesh = Mesh(np.asarray(devices), ("core",))
    spec = PartitionSpec("core")
    fn = jax.jit(
        shard_map(
            _body,
            mesh=mesh,
            in_specs=(spec,) * (n_params + len(out_names)),
            out_specs=(spec,) * len(out_names),
            check_rep=False,
        ),
        keep_unused=True,
    )

    def put(arr):
        return jax.device_put(arr, NamedSharding(mesh, spec))

    return fn, put, in_names, out_names



# revision 2
# speedup vs baseline: 14.8141x; 14.8141x over previous
"""Trainium2 Bass kernel for FullTensorProduct (64x0e+64x1o) x (1x0e+1x1o), uvuv.

Self-contained: accepts FULL inputs x1 (100000, 256) f32 and x2 (100000, 4)
f32, shards the edge dimension across 8 NeuronCores (pure data parallel),
runs one SPMD Bass kernel, and returns the FULL (100000, 1024) f32 output.

Per-core layout: 12500 edges mapped group-major as edge = t*125 + p over
125 SBUF partitions x 100 groups, so each supertile's x1/out DRAM region is
contiguous; x2 is host-permuted to match. Supertiles of G=10 groups.

Key measured optimizations over the f32 / ScalarE-heavy baseline
(per-pass slope on HW, For_i(0,33) loop, 438us -> 251us):
- out stored bf16 (compute stays f32; final ops cast on write). Store
  traffic halves: 51.2 -> 25.6 MB/core. rel err 2.2e-3 vs 2e-2 gate.
- all DMAs on the gpsimd SWDGE ring, issued as 2-group chunks: measured
  233GB/s aggregate vs 185GB/s at 10-group granularity (HWDGE rings are
  slower, ~130GB/s, and do NOT run in parallel with SWDGE).
- Pool engine kept free for SWDGE descriptor-gen: ALL elementwise compute
  on DVE (products, sv, ss, vs, cross, quad, A/B/C) except the two
  constant scales (vv0, q2) on ScalarE. Putting broadcast muls on Pool
  (v5) or per-group muls on ScalarE (baseline) measured far slower.
- xin bufs=4 / outp bufs=3 double-buffering depth (250.6us vs 270.8 at 3/2).

Output columns (1024): [o_ss 0:64 | o_vv0 64:128 | o_sv 128:320 |
                        o_vs 320:512 | o_cross 512:704 | o_quad 704:1024]
"""

import numpy as np

import concourse.bass as bass
import concourse.bacc as bacc
import concourse.mybir as mybir
import concourse.tile as tile
from concourse.bass_utils import run_bass_kernel_spmd

F32 = mybir.dt.float32
BF16 = mybir.dt.bfloat16
INV_SQRT2 = float(1.0 / np.sqrt(2.0))
C_VV0 = float(np.sqrt(2.0) / np.sqrt(3.0))
C_Q2 = float(1.0 / np.sqrt(3.0))

N_CORES = 8
N_EDGES = 100000
ROWS_PER_CORE = N_EDGES // N_CORES  # 12500
P_PART = 125                        # 12500 = 125 * 100
G_GROUPS = 10


def _emit(nc: bass.Bass, rows: int, P: int, G: int, n_reps: int = 1, gc: int = 2):
    T = rows // P
    assert P * T == rows and T % G == 0 and G % gc == 0
    n_super = T // G
    mult = mybir.AluOpType.mult
    subtract = mybir.AluOpType.subtract

    x1 = nc.dram_tensor("x1", (rows, 256), F32, kind="ExternalInput")
    x2 = nc.dram_tensor("x2", (rows, 4), F32, kind="ExternalInput")
    out = nc.dram_tensor("out", (rows, 1024), BF16, kind="ExternalOutput")

    # group-major: edge = t*P + p, so every supertile's x1/out DRAM region
    # is one contiguous block. x2 arrives host-permuted to [p*T+t] order.
    X1 = x1[:].rearrange("(t p) c -> p t c", p=P)
    X2 = x2[:].rearrange("(p t) c -> p (t c)", p=P)
    OUT = out[:].rearrange("(t p) c -> p t c", p=P)

    with tile.TileContext(nc) as tc:
        with (
            tc.tile_pool(name="xin", bufs=4) as xin_pool,
            tc.tile_pool(name="outp", bufs=3) as out_pool,
            tc.tile_pool(name="prod", bufs=2) as prod_pool,
            tc.tile_pool(name="singles", bufs=1) as singles,
        ):
            x2t = singles.tile([P, T * 4], F32)
            x2s = singles.tile([P, T * 4], F32)
            nc.sync.dma_start(out=x2t[:], in_=X2)
            nc.vector.tensor_scalar_mul(out=x2s[:], in0=x2t[:], scalar1=INV_SQRT2)
            x2s3 = x2s[:].rearrange("p (t c) -> p t c", c=4)
            x2r3 = x2t[:].rearrange("p (t c) -> p t c", c=4)

            def one_pass():
                for s in range(n_super):
                    t0 = s * G
                    xt = xin_pool.tile([P, G, 256], F32, tag="x1t")
                    for c in range(0, G, gc):
                        nc.gpsimd.dma_start(
                            out=xt[:, c : c + gc, :],
                            in_=X1[:, t0 + c : t0 + c + gc, :],
                        )
                    ot = out_pool.tile([P, G, 1024], BF16, tag="outt")

                    # products P_i[p,g,u,j] = v1[u,i] * v2[j] / sqrt2  (DVE)
                    v1 = xt[:, :, 64:256].rearrange("p g (u i) -> p g u i", i=3)
                    v2b = (
                        x2s3[:, t0 : t0 + G, 1:4]
                        .unsqueeze(2)
                        .broadcast_to((P, G, 64, 3))
                    )
                    Pt = []
                    for i in range(3):
                        pt = prod_pool.tile([P, G, 64, 3], F32, tag=f"P{i}")
                        in0 = v1[:, :, :, i].unsqueeze(3).broadcast_to((P, G, 64, 3))
                        nc.vector.tensor_mul(out=pt[:], in0=in0, in1=v2b)
                        Pt.append(pt)

                    def pij(i, j):
                        return Pt[i][:, :, :, j]

                    # o_sv / o_ss / o_vs as broadcast TT ops on DVE (bf16 out)
                    svd = ot[:, :, 128:320].rearrange("p g (u j) -> p g u j", j=3)
                    s1b = xt[:, :, 0:64].unsqueeze(3).broadcast_to((P, G, 64, 3))
                    v2raw = (
                        x2r3[:, t0 : t0 + G, 1:4]
                        .unsqueeze(2)
                        .broadcast_to((P, G, 64, 3))
                    )
                    nc.vector.tensor_mul(out=svd, in0=s1b, in1=v2raw)
                    s2b64 = x2r3[:, t0 : t0 + G, 0:1].broadcast_to((P, G, 64))
                    s2b192 = x2r3[:, t0 : t0 + G, 0:1].broadcast_to((P, G, 192))
                    nc.vector.tensor_mul(out=ot[:, :, 0:64], in0=xt[:, :, 0:64], in1=s2b64)
                    nc.vector.tensor_mul(
                        out=ot[:, :, 320:512], in0=xt[:, :, 64:256], in1=s2b192
                    )

                    # cross / quad combines (DVE)
                    crs = ot[:, :, 512:704].rearrange("p g (u k) -> p g u k", k=3)
                    qd = ot[:, :, 704:1024].rearrange("p g (u m) -> p g u m", m=5)
                    nc.vector.tensor_sub(out=crs[:, :, :, 0], in0=pij(1, 2), in1=pij(2, 1))
                    nc.vector.tensor_sub(out=crs[:, :, :, 1], in0=pij(2, 0), in1=pij(0, 2))
                    nc.vector.tensor_sub(out=crs[:, :, :, 2], in0=pij(0, 1), in1=pij(1, 0))
                    nc.vector.tensor_add(out=qd[:, :, :, 0], in0=pij(0, 1), in1=pij(1, 0))
                    nc.vector.tensor_add(out=qd[:, :, :, 1], in0=pij(1, 2), in1=pij(2, 1))
                    nc.vector.tensor_add(out=qd[:, :, :, 3], in0=pij(0, 2), in1=pij(2, 0))
                    nc.vector.tensor_sub(out=qd[:, :, :, 4], in0=pij(0, 0), in1=pij(1, 1))

                    # vv0 / q2 chain: adds on DVE, const scales on ScalarE
                    A = prod_pool.tile([P, G, 64], F32, tag="A")
                    B = prod_pool.tile([P, G, 64], F32, tag="B")
                    C = prod_pool.tile([P, G, 64], F32, tag="C")
                    nc.vector.tensor_add(out=A[:], in0=pij(0, 0), in1=pij(1, 1))
                    nc.vector.tensor_add(out=B[:], in0=A[:], in1=pij(2, 2))
                    nc.scalar.mul(out=ot[:, :, 64:128], in_=B[:], mul=C_VV0)
                    nc.vector.scalar_tensor_tensor(
                        out=C[:], in0=pij(2, 2), scalar=3.0, in1=B[:],
                        op0=mult, op1=subtract,
                    )
                    nc.scalar.mul(out=qd[:, :, :, 2], in_=C[:], mul=C_Q2)

                    for c in range(0, G, gc):
                        nc.gpsimd.dma_start(
                            out=OUT[:, t0 + c : t0 + c + gc, :],
                            in_=ot[:, c : c + gc, :],
                        )

            if n_reps == 1:
                one_pass()
            else:
                with tc.For_i(0, n_reps):
                    one_pass()
    return nc


_NC_CACHE = None


def _get_nc() -> bass.Bass:
    global _NC_CACHE
    if _NC_CACHE is None:
        nc = bacc.Bacc()
        _emit(nc, ROWS_PER_CORE, P_PART, G_GROUPS)
        nc.compile()   # bacc: reg alloc + split multi-sync-waits for TRN2
        nc.finalize()
        _NC_CACHE = nc
    return _NC_CACHE


def run(x1: np.ndarray, x2: np.ndarray, trace: bool = False):
    """Shard, run SPMD on 8 cores, gather. Returns (out, BassKernelResults)."""
    assert x1.shape == (N_EDGES, 256) and x2.shape == (N_EDGES, 4)
    x1 = np.ascontiguousarray(x1, dtype=np.float32)
    x2 = np.ascontiguousarray(x2, dtype=np.float32)
    nc = _get_nc()
    R = ROWS_PER_CORE
    T = R // P_PART
    perm = (np.arange(T)[None, :] * P_PART + np.arange(P_PART)[:, None]).reshape(-1)
    in_maps = [
        {"x1": x1[i * R : (i + 1) * R], "x2": x2[i * R : (i + 1) * R][perm]}
        for i in range(N_CORES)
    ]
    br = run_bass_kernel_spmd(nc, in_maps, list(range(N_CORES)), trace=trace)
    out = np.concatenate(
        [np.asarray(br.results[i]["out"]).astype(np.float32) for i in range(N_CORES)],
        axis=0,
    )
    return out, br


def kernel(x1: np.ndarray, x2: np.ndarray) -> np.ndarray:
    out, _ = run(x1, x2, trace=False)
    return out


def make_timed_runner(nc=None, n_cores=N_CORES):
    """Build a shard_map-jitted callable over the 8 cores with device-resident
    inputs, mirroring bass2jax.run_bass_via_pjrt but without output donation,
    so repeated invocations measure device execution time.

    Returns (fn, put, in_names, out_names): call fn(*args) -> jax outputs.
    """
    import jax
    import jax.numpy as jnp
    from jax.experimental.shard_map import shard_map
    from jax.sharding import Mesh, NamedSharding, PartitionSpec

    from concourse import bass2jax, mybir as mb

    bass2jax.install_neuronx_cc_hook()
    if nc is None:
        nc = _get_nc()
    assert nc.dbg_addr is None
    partition_name = nc.partition_id_tensor.name if nc.partition_id_tensor else None

    in_names, out_names, out_avals = [], [], []
    for alloc in nc.m.functions[0].allocations:
        if not isinstance(alloc, mb.MemoryLocationSet):
            continue
        name = alloc.memorylocations[0].name
        if alloc.kind == "ExternalInput":
            if name != partition_name:
                in_names.append(name)
        elif alloc.kind == "ExternalOutput":
            out_names.append(name)
            out_avals.append(
                jax.core.ShapedArray(tuple(alloc.tensor_shape), mb.dt.np(alloc.dtype))
            )
    n_params = len(in_names)
    all_names = in_names + out_names
    if partition_name is not None:
        all_names = all_names + [partition_name]

    def _body(*args):
        operands = list(args)
        if partition_name is not None:
            operands.append(bass2jax.partition_id_tensor())
        outs = bass2jax._bass_exec_p.bind(
            *operands,
            out_avals=tuple(out_avals),
            in_names=tuple(all_names),
            out_names=tuple(out_names),
            lowering_input_output_aliases=(),
            sim_require_finite=True,
            sim_require_nnan=True,
            nc=nc,
        )
        return tuple(outs)

    devices = jax.devices()[:n_cores]
    mesh = Mesh(np.asarray(devices), ("core",))
    spec = PartitionSpec("core")
    fn = jax.jit(
        shard_map(
            _body,
            mesh=mesh,
            in_specs=(spec,) * (n_params + len(out_names)),
            out_specs=(spec,) * len(out_names),
            check_rep=False,
        ),
        keep_unused=True,
    )

    def put(arr):
        return jax.device_put(arr, NamedSharding(mesh, spec))

    return fn, put, in_names, out_names
